# revision 32
# baseline (speedup 1.0000x reference)
"""Multi-head causal attention (B=4, T=2048, C=1024, H=16) on 8 trn2 cores.

Sharding: data-parallel over batch (4) x sequence-parallel over causal query
blocks (2), zig-zag balanced so all 8 cores run one identical program:
  core = 2*b + half;  half 0 gets query blocks [0,2,4,6,9,11,13,15],
  half 1 gets [1,3,5,7,8,10,12,14].  Slot s (0..7) processes J(s)=2s+2 key
  blocks; causal boundary handled by per-core input masks on the last two.
Each core writes a disjoint [1024, 1024] slice of the output; the host
scatters slices back and adds the (v/o-bias) correction  bo + bv @ Wo.T.

Schedule: after the Q projection, everything runs as ONE fused stream of
fine-grained units — per-head-pair K-projection chains, score psum-groups
(exp'd by the Scalar engine as they land), bank-interleaved attnV chains,
V-projection chains and attn_out transposes woven in as PE filler, ending
with the output projection staged through the dead attn_out tile.  This
keeps the PE array's duty cycle ~96% so the HAM clock gate never
re-throttles it to 1.2 GHz (the gate re-throttles on fractional idleness
of a free-running 3.4us window, and un-throttling needs ~3.4us of
uninterrupted busy — one coarse phase boundary can halve the clock for the
rest of the kernel).  DMAs are issued in exact consumption order (startup
is HBM-bw bound with all 8 cores loading at once); wv/wo ride rotating
half-buffers to fit SBUF.  exp is Scalar-only (~165us/core); the PE
(~307us busy at 2.4 GHz) is the critical engine, with attnV matmuls
LDWEIGHTS-bound (a fresh 128x128 stationary per 65-row stream).
"""

import numpy as np
import ml_dtypes

import concourse.bass as bass
import concourse.mybir as mybir
import concourse.tile as tile
from concourse import bacc
from concourse.bass import ts
from concourse.bass_utils import run_bass_kernel_spmd

B, T, C, H, DK = 4, 2048, 1024, 16, 64
P = 128
NB = T // P          # 16 key blocks
SLOTS = 8            # query blocks per core
SCALE = 1.0 / np.sqrt(DK)
BF16 = mybir.dt.bfloat16
F32 = mybir.dt.float32
F32R = mybir.dt.float32r
FP8 = mybir.dt.float8e4
EXP = mybir.ActivationFunctionType.Exp

QBLKS = [
    [0, 2, 4, 6, 9, 11, 13, 15],
    [1, 3, 5, 7, 8, 10, 12, 14],
]

_cache = {}


# packed expS layout: row jb stores queries [q0(jb), 1024) where
# q0(jb) = 128*(jb//2); OFF[jb] is the packed column offset.
W_JB = [T // 2 - P * (jb // 2) for jb in range(NB)]
OFF_JB = [0] * NB
for _jb in range(1, NB):
    OFF_JB[_jb] = OFF_JB[_jb - 1] + W_JB[_jb - 1]
NPACK = OFF_JB[-1] + W_JB[-1]  # 9216


def _build():
    nc = bacc.Bacc("TRN2", target_bir_lowering=False, debug=False)

    xT = nc.dram_tensor("xT", [C, T], BF16, kind="ExternalInput").ap()
    xTq = nc.dram_tensor("xTq", [C, SLOTS * P], BF16, kind="ExternalInput").ap()
    wqT = nc.dram_tensor("wqT", [C, C], BF16, kind="ExternalInput").ap()
    wkT = nc.dram_tensor("wkT", [C, C], BF16, kind="ExternalInput").ap()
    wvT = nc.dram_tensor("wvT", [C, C], BF16, kind="ExternalInput").ap()
    woT = nc.dram_tensor("woT", [C, C], BF16, kind="ExternalInput").ap()
    bq = nc.dram_tensor("bq", [P, C // P], F32, kind="ExternalInput").ap()
    bk = nc.dram_tensor("bk", [P, C // P], F32, kind="ExternalInput").ap()
    masks = nc.dram_tensor("masks", [SLOTS, 2, P, P], FP8, kind="ExternalInput").ap()
    ident = nc.dram_tensor("ident", [P, P], BF16, kind="ExternalInput").ap()
    y = nc.dram_tensor("y", [SLOTS * P, C], BF16, kind="ExternalOutput").ap()

    CB = C // P  # 8 column blocks of the channel dim

    with tile.TileContext(nc) as tc:
        with (
            tc.tile_pool(name="const", bufs=1) as cpool,
            tc.tile_pool(name="attn", bufs=1) as apool,
        ):
            masks_sb = cpool.tile([P, SLOTS, 2, P], FP8)
            ident_sb = cpool.tile([P, P], BF16)
            bq_sb = cpool.tile([P, CB], F32)
            bk_sb = cpool.tile([P, CB], F32)

            attn_out = apool.tile([P, SLOTS, C], BF16)

            with tc.tile_pool(name="qkv", bufs=1) as qkv:
                qT = qkv.tile([P, CB, SLOTS * P], BF16)
                kT = qkv.tile([P, CB, T], BF16)
                v = qkv.tile([P, NB, H * (DK + 1)], BF16)
                vg = v[:].rearrange("p t (h e) -> p t h e", e=DK + 1)
                nc.vector.memset(vg[:, :, :, DK : DK + 1], 1.0)

                # ---- Q projection (kb-major, 8 psum banks) ----
                with (
                    tc.tile_pool(name="xq", bufs=1) as xq_pool,
                    tc.tile_pool(name="wq", bufs=1) as wq_pool,
                    tc.tile_pool(name="pq", bufs=1, space="PSUM") as pq,
                ):
                    nc.gpsimd.dma_start(bq_sb[:], bq[:])
                    nc.gpsimd.dma_start(bk_sb[:], bk[:])
                    xq_sb = xq_pool.tile([P, CB, SLOTS * P], BF16)
                    xTq_r = xTq.rearrange("(ko p) t -> p ko t", p=P)
                    wq_sb = wq_pool.tile([P, CB, C], BF16)
                    wq_r = wqT.rearrange("(ko p) n -> p ko n", p=P)
                    # startup is HBM-bw bound with all 8 cores loading at
                    # once: per-kb xq/wq pairs land just-in-time for the
                    # kb-major Q loop, so the PE starts ~7us earlier
                    for kb in range(CB):
                        nc.gpsimd.dma_start(xq_sb[:, kb, :], xTq_r[:, kb, :])
                        nc.gpsimd.dma_start(wq_sb[:, kb, :], wq_r[:, kb, :])
                    for nch in range(2):
                        acc = [
                            pq.tile([P, 512], F32, tag=f"qacc{cb}", name=f"qacc{cb}")
                            for cb in range(CB)
                        ]
                        for kb in range(CB):
                            for cb in range(CB):
                                nc.tensor.matmul(
                                    acc[cb][:],
                                    wq_sb[:, kb, ts(cb, P)],
                                    xq_sb[:, kb, ts(nch, 512)],
                                    start=(kb == 0),
                                    stop=(kb == CB - 1),
                                )
                        for cb in range(CB):
                            nc.vector.tensor_scalar_add(
                                qT[:, cb, ts(nch, 512)], acc[cb][:], bq_sb[:, cb : cb + 1]
                            )

                # ---- fused stream: K-proj / scores / exp / attnV / V-proj ----
                # Work is emitted as fine-grained units: score psum-groups
                # (Scalar-paced via exp) woven with K/V projection chains and
                # attnV chains, so the PE never idles long enough for the HAM
                # clock gate to re-throttle it, and the Scalar engine always
                # has the next score tile ready.
                with (
                    tc.tile_pool(name="xt", bufs=1) as xt_pool,
                    tc.tile_pool(name="wres", bufs=1) as wres,
                    tc.tile_pool(name="expS", bufs=2) as spool,
                    tc.tile_pool(name="small", bufs=4) as small,
                    tc.tile_pool(name="atp", bufs=1) as atp,
                    tc.tile_pool(name="pp", bufs=2, space="PSUM") as pp,
                    tc.tile_pool(name="ps_s", bufs=2, space="PSUM") as ps_s,
                    tc.tile_pool(name="ps_o", bufs=2, space="PSUM") as ps_o,
                ):
                    xT_sb = xt_pool.tile([P, CB, T], BF16)
                    wk_sb = wres.tile([P, CB, C], BF16, tag="wk", name="wk")
                    wv0_sb = wres.tile([P, CB, 512], BF16, tag="wv", name="wv0")
                    xT_r = xT.rearrange("(ko p) t -> p ko t", p=P)
                    wk_r = wkT.rearrange("(ko p) n -> p ko n", p=P)
                    wv_r = wvT.rearrange("(ko p) n -> p ko n", p=P)
                    # K-proj's first inputs go through the idle Scalar
                    # engine's DMA queue so they race the Q-phase loads
                    # instead of queuing behind them (startup is HBM-bw
                    # bound; the PE reaches K ~30us after Q starts).
                    nc.scalar.dma_start(wk_sb[:, :, 0:128], wk_r[:, :, 0:128])
                    nc.scalar.dma_start(xT_sb[:, :, 0:512], xT_r[:, :, 0:512])
                    nc.gpsimd.dma_start(xT_sb[:, :, 512:1024], xT_r[:, :, 512:1024])
                    nc.gpsimd.dma_start(wk_sb[:, :, 128:512], wk_r[:, :, 128:512])
                    nc.gpsimd.dma_start(wk_sb[:, :, 512:1024], wk_r[:, :, 512:1024])
                    nc.gpsimd.dma_start(
                        xT_sb[:, :, 1024:2048], xT_r[:, :, 1024:2048]
                    )
                    nc.gpsimd.dma_start(
                        masks_sb[:], masks[:].rearrange("s t p q -> p s t q")
                    )
                    nc.gpsimd.dma_start(ident_sb[:], ident[:])
                    nc.gpsimd.dma_start(wv0_sb[:], wv_r[:, :, 0:512])

                    exp_tiles = {}

                    def u_kchain(cb, nch):
                        def emit():
                            acc = pp.tile([P, 512], F32, tag="pp")
                            for kb in range(CB):
                                nc.tensor.matmul(
                                    acc[:],
                                    wk_sb[:, kb, ts(cb, P)],
                                    xT_sb[:, kb, ts(nch, 512)],
                                    start=(kb == 0),
                                    stop=(kb == CB - 1),
                                )
                            nc.vector.tensor_scalar_add(
                                kT[:, cb, ts(nch, 512)], acc[:], bk_sb[:, cb : cb + 1]
                            )
                        return emit

                    def u_vchain(tb, dch, wv_half):
                        def emit():
                            acc = pp.tile([P, 512], F32, tag="pp")
                            for kb in range(CB):
                                nc.tensor.matmul(
                                    acc[:],
                                    xT_sb[:, kb, ts(tb, P)],
                                    wv_half[:, kb, :],
                                    start=(kb == 0),
                                    stop=(kb == CB - 1),
                                )
                            nc.vector.tensor_copy(
                                vg[:, tb, dch * 8 : (dch + 1) * 8, 0:DK],
                                acc[:].rearrange("p (h e) -> p h e", e=DK),
                            )
                        return emit

                    def u_transpose(cb):
                        # real late-slot filler: transpose the finished
                        # head-pair column of attn_out into aT (PSUM via the
                        # shared pp tag, bitcast to bf16)
                        def emit():
                            for s in range(SLOTS):
                                pst = pp.tile([P, 512], F32, tag="pp", name="pst")
                                pb = pst[:, 0:64].bitcast(BF16)
                                nc.tensor.transpose(
                                    pb, attn_out[:, s, ts(cb, P)], ident_sb[:]
                                )
                                nc.vector.tensor_copy(aT[:, cb, ts(s, P)], pb)
                        return emit

                    def u_score(h, jb):
                        # one jb-pair group: matmuls + exp (+ causal mask)
                        hp = (h % 2) * DK
                        cbh = h // 2
                        w = W_JB[jb]
                        q0 = P * (jb // 2)
                        s = jb // 2

                        def emit():
                            expS = exp_tiles[h]
                            if jb >= 8:
                                pss = ps_s.tile([P, SLOTS * P], F32, tag="pss")
                                for i in range(2):
                                    nc.tensor.matmul(
                                        pss[:, 512 * i : 512 * i + w],
                                        kT[hp : hp + DK, cbh, ts(jb + i, P)],
                                        qT[hp : hp + DK, cbh, q0:],
                                        start=True,
                                        stop=True,
                                    )
                                nc.scalar.activation(
                                    expS[
                                        :, OFF_JB[jb] : OFF_JB[jb] + 2 * w
                                    ].rearrange("p (t q) -> p t q", t=2),
                                    pss[:].rearrange("p (t c) -> p t c", t=2)[
                                        :, :, 0:w
                                    ],
                                    EXP,
                                    scale=float(SCALE),
                                )
                            else:
                                for i in range(2):
                                    pss = ps_s.tile([P, SLOTS * P], F32, tag="pss")
                                    for aa, bb in ((q0, 512), (512, SLOTS * P)):
                                        nc.tensor.matmul(
                                            pss[:, aa:bb],
                                            kT[hp : hp + DK, cbh, ts(jb + i, P)],
                                            qT[hp : hp + DK, cbh, aa:bb],
                                            start=True,
                                            stop=True,
                                        )
                                    nc.scalar.activation(
                                        expS[:, OFF_JB[jb + i] : OFF_JB[jb + i] + w],
                                        pss[:, q0:],
                                        EXP,
                                        scale=float(SCALE),
                                    )
                            blk = expS[:, OFF_JB[jb] : OFF_JB[jb] + 2 * w].rearrange(
                                "p (t q) -> p t q", t=2
                            )[:, :, 0:P]
                            nc.vector.tensor_mul(blk, blk, masks_sb[:, s, :, :])
                        return emit

                    def u_attnv(h):
                        # identical matmul order to the solid-block version,
                        # returned as ~6-mm chunks so callers can interleave
                        # projection chains between them (their 512-row
                        # streams hide the 128x128 expS stationary loads).
                        expS = exp_tiles[h]
                        pso0 = ps_o.tile([P, 4, DK + 1], F32, tag="pso", name="pso0")
                        pso1 = ps_o.tile([P, 4, DK + 1], F32, tag="pso", name="pso1")
                        vcol = v[:, :, h * (DK + 1) : (h + 1) * (DK + 1)]
                        mms = []
                        for si in range(4):
                            sA, sB = si, si + 4
                            JA, JB = 2 * sA + 2, 2 * sB + 2
                            for jb in range(JB):
                                if jb < JA:
                                    mms.append((pso0, si, sA, jb, JA))
                                mms.append((pso1, si, sB, jb, JB))
                        chunks = []
                        for i in range(0, len(mms), 6):

                            def emit(chunk=mms[i : i + 6]):
                                for pso, si, s, jb, J in chunk:
                                    o = OFF_JB[jb] + s * P - P * (jb // 2)
                                    nc.tensor.matmul(
                                        pso[:, si, :],
                                        expS[:, o : o + P],
                                        vcol[:, jb, :],
                                        start=(jb == 0),
                                        stop=(jb == J - 1),
                                    )

                            chunks.append(emit)

                        def norms():
                            for g, pso in ((0, pso0), (1, pso1)):
                                rec = small.tile([P, 4], F32, tag="rec")
                                nc.vector.reciprocal(
                                    rec[:],
                                    pso[:, :, DK : DK + 1].rearrange(
                                        "p s o -> p (s o)"
                                    ),
                                )
                                for si in range(4):
                                    s = 4 * g + si
                                    nc.vector.tensor_scalar_mul(
                                        attn_out[:, s, h * DK : (h + 1) * DK],
                                        pso[:, si, 0:DK],
                                        rec[:, si : si + 1],
                                    )

                        chunks.append(norms)
                        return chunks

                    def run_block(ach, units):
                        # interleave attnV chunks between streaming units
                        na, nu = len(ach), len(units)
                        ai = 0
                        for i, u in enumerate(units):
                            u()
                            while ai * nu < (i + 1) * na:
                                ach[ai]()
                                ai += 1
                        for a in ach[ai:]:
                            a()

                    def new_head(h):
                        exp_tiles[h] = spool.tile([P, NPACK], BF16, tag="expS", name="expS")

                    # V chains: dch0 fills slot 0 (A(0) needs it at
                    # slot 1), dch1 fills slots 2-4 (deadline: A(8) at slot
                    # 5; slot 1 left light so the wv1 load can land).
                    # Slots 5-7 are filled with the attn_out transposes.
                    aT = atp.tile([P, CB, SLOTS * P], BF16)
                    vfill = {0: [u_vchain(tb, 0, wv0_sb) for tb in range(NB)]}

                    for c in range(CB):
                        h0, h1 = 2 * c, 2 * c + 1
                        if c == 1:
                            wv1_sb = wres.tile(
                                [P, CB, 512], BF16, tag="wv", name="wv1"
                            )
                            nc.gpsimd.dma_start(wv1_sb[:], wv_r[:, :, 512:1024])
                            vfill[2] = [u_vchain(tb, 1, wv1_sb) for tb in range(6)]
                            vfill[3] = [u_vchain(tb, 1, wv1_sb) for tb in range(6, 11)]
                            vfill[4] = [u_vchain(tb, 1, wv1_sb) for tb in range(11, 16)]
                            vfill[5] = [u_transpose(cb) for cb in range(4)]
                            vfill[6] = [u_transpose(4)]
                            vfill[7] = [u_transpose(5)]
                        fills = list(vfill.get(c, []))
                        pre = [u_kchain(c, 0)]
                        while fills and len(pre) < 4:
                            pre.append(fills.pop(0))
                        mid = []
                        while fills and len(mid) < 3:
                            mid.append(fills.pop(0))
                        if c >= 1:
                            run_block(u_attnv(h0 - 2), pre)
                        else:
                            for u in pre:
                                u()
                        new_head(h0)
                        nf = len(fills)
                        fi = 0
                        for p in range(8):  # jb-pair index
                            if p % 2 == 0 and p > 0:
                                u_kchain(c, p // 2)()
                            u_score(h0, 2 * p)()
                            while fi * 16 < (p + 1) * nf:
                                fills[fi]()
                                fi += 1
                        if c >= 1:
                            run_block(u_attnv(h0 - 1), mid)
                        else:
                            for u in mid:
                                u()
                        new_head(h1)
                        for p in range(8):
                            u_score(h1, 2 * p)()
                            while fi * 16 < (p + 9) * nf:
                                fills[fi]()
                                fi += 1
                        for f in fills[fi:]:
                            f()
                    run_block(u_attnv(H - 2), [u_transpose(6)])
                    woT_sb = wres.tile([P, CB, C], BF16, tag="wk", name="woT")
                    woT_r = woT.rearrange("(ko p) n -> p ko n", p=P)
                    nc.gpsimd.dma_start(woT_sb[:, :, 0:512], woT_r[:, :, 0:512])
                    nc.gpsimd.dma_start(
                        woT_sb[:, :, 512:1024], woT_r[:, :, 512:1024]
                    )
                    for t in u_attnv(H - 1):
                        t()
                    u_transpose(7)()

                    # ---- output projection, staged through the (now dead)
                    # attn_out tile in bf16; y is written back as bf16 ----
                    y_r = y.rearrange("(tb p) c -> p tb c", p=P)
                    for tb in range(SLOTS):
                        for nch in range(2):
                            psy = pp.tile([P, 512], F32, tag="pp", name="psy")
                            for cbk in range(CB):
                                nc.tensor.matmul(
                                    psy[:],
                                    aT[:, cbk, ts(tb, P)],
                                    woT_sb[:, cbk, ts(nch, 512)],
                                    start=(cbk == 0),
                                    stop=(cbk == CB - 1),
                                )
                            st = attn_out[:, tb, 512 * nch : 512 * nch + 512]
                            nc.vector.tensor_copy(st, psy[:])
                            nc.gpsimd.dma_start(y_r[:, tb, ts(nch, 512)], st)

    nc.compile()
    return nc


def _host_inputs(x, mask, Wq, bq_v, Wk, bk_v, Wv, bv_v, Wo, bo_v):
    """Per-core input maps + the host-side output bias correction."""
    f32 = np.float32
    bf16 = ml_dtypes.bfloat16
    wqT = np.ascontiguousarray(np.asarray(Wq, f32).T).astype(bf16)
    wkT = np.ascontiguousarray(np.asarray(Wk, f32).T).astype(bf16)
    wvT = np.ascontiguousarray(np.asarray(Wv, f32).T).astype(bf16)
    woT = np.ascontiguousarray(np.asarray(Wo, f32).T).astype(bf16)
    bq_p = np.ascontiguousarray(np.asarray(bq_v, f32).reshape(C // P, P).T)
    bk_p = np.ascontiguousarray(np.asarray(bk_v, f32).reshape(C // P, P).T)
    identity = np.eye(P, dtype=f32).astype(bf16)
    # exact v/o bias fold: softmax rows sum to 1, so v+bv adds bv to attn out
    bo_eff = (np.asarray(bo_v, f32) + np.asarray(bv_v, f32) @ np.asarray(Wo, f32).T)

    # per-half causal boundary masks for the last two key blocks of each slot
    mask_half = []
    tri = np.tril(np.ones((P, P), f32)).T  # [j, i] = 1 where j <= i
    for half in range(2):
        m = np.zeros((SLOTS, 2, P, P), f32)
        for s in range(SLOTS):
            g = QBLKS[half][s]
            for idx, jb in enumerate((2 * s, 2 * s + 1)):
                if jb < g:
                    m[s, idx] = 1.0
                elif jb == g:
                    m[s, idx] = tri
        mask_half.append(m.astype(ml_dtypes.float8_e4m3fn))

    xn = np.asarray(x, f32)
    in_maps = []
    for core in range(8):
        b, half = divmod(core, 2)
        xT = np.ascontiguousarray(xn[b].T).astype(bf16)
        qtok = np.concatenate([np.arange(g * P, (g + 1) * P) for g in QBLKS[half]])
        xTq = np.ascontiguousarray(xn[b][qtok].T).astype(bf16)
        in_maps.append(
            {
                "xT": xT,
                "xTq": xTq,
                "wqT": wqT,
                "wkT": wkT,
                "wvT": wvT,
                "woT": woT,
                "bq": bq_p,
                "bk": bk_p,
                "masks": mask_half[half],
                "ident": identity,
            }
        )
    return in_maps, bo_eff


def _run(inputs, trace=False):
    if "nc" not in _cache:
        _cache["nc"] = _build()
    nc = _cache["nc"]
    in_maps, bo_eff = _host_inputs(
        inputs["x"], inputs["mask"],
        inputs["Wq"], inputs["bq"], inputs["Wk"], inputs["bk"],
        inputs["Wv"], inputs["bv"], inputs["Wo"], inputs["bo"],
    )
    res = run_bass_kernel_spmd(nc, in_maps, list(range(8)), trace=trace)
    out = np.empty((B, T, C), np.float32)
    for core in range(8):
        b, half = divmod(core, 2)
        yc = np.asarray(res.results[core]["y"], dtype=np.float32)
        for s, g in enumerate(QBLKS[half]):
            out[b, g * P : (g + 1) * P] = yc[s * P : (s + 1) * P]
    out += bo_eff
    return out, res


def kernel(**inputs):
    out, _ = _run(inputs, trace=False)
    return out



# revision 34
# speedup vs baseline: 1.2286x; 1.2286x over previous
"""Multi-head causal attention (B=4, T=2048, C=1024, H=16) on 8 trn2 cores.

Sharding: data-parallel over batch (4) x sequence-parallel over causal query
blocks (2), zig-zag balanced so all 8 cores run one identical program:
  core = 2*b + half;  half 0 gets query blocks [0,2,4,6,9,11,13,15],
  half 1 gets [1,3,5,7,8,10,12,14].  Slot s (0..7) processes J(s)=2s+2 key
  blocks; causal boundary handled by per-core input masks on the last two.
Each core writes a disjoint [1024, 1024] slice of the output; the host
scatters slices back and adds the (v/o-bias) correction  bo + bv @ Wo.T.

Schedule: after the Q projection, everything runs as ONE fused stream of
fine-grained units — per-head-pair K-projection chains, score psum-groups
(exp'd by the Scalar engine as they land), bank-interleaved attnV chains,
V-projection chains and attn_out transposes woven in as PE filler, ending
with the output projection staged through the dead attn_out tile.  This
keeps the PE array's duty cycle ~96% so the HAM clock gate never
re-throttles it to 1.2 GHz (the gate re-throttles on fractional idleness
of a free-running 3.4us window, and un-throttling needs ~3.4us of
uninterrupted busy — one coarse phase boundary can halve the clock for the
rest of the kernel).  DMAs are issued in exact consumption order (startup
is HBM-bw bound with all 8 cores loading at once); wv/wo ride rotating
half-buffers to fit SBUF.  exp is Scalar-only (~165us/core); the PE
(~307us busy at 2.4 GHz) is the critical engine, with attnV matmuls
LDWEIGHTS-bound (a fresh 128x128 stationary per 65-row stream).
"""

import numpy as np
import ml_dtypes

import concourse.bass as bass
import concourse.mybir as mybir
import concourse.tile as tile
from concourse import bacc
from concourse.bass import ts
from concourse.bass_utils import run_bass_kernel_spmd

B, T, C, H, DK = 4, 2048, 1024, 16, 64
P = 128
NB = T // P          # 16 key blocks
SLOTS = 8            # query blocks per core
SCALE = 1.0 / np.sqrt(DK)
BF16 = mybir.dt.bfloat16
F32 = mybir.dt.float32
F32R = mybir.dt.float32r
FP8 = mybir.dt.float8e4
EXP = mybir.ActivationFunctionType.Exp

QBLKS = [
    [0, 2, 4, 6, 9, 11, 13, 15],
    [1, 3, 5, 7, 8, 10, 12, 14],
]

_cache = {}


# packed expS layout: row jb stores queries [q0(jb), 1024) where
# q0(jb) = 128*(jb//2); OFF[jb] is the packed column offset.
W_JB = [T // 2 - P * (jb // 2) for jb in range(NB)]
OFF_JB = [0] * NB
for _jb in range(1, NB):
    OFF_JB[_jb] = OFF_JB[_jb - 1] + W_JB[_jb - 1]
NPACK = OFF_JB[-1] + W_JB[-1]  # 9216


def _build():
    nc = bacc.Bacc("TRN2", target_bir_lowering=False, debug=False)

    xT = nc.dram_tensor("xT", [C, T], BF16, kind="ExternalInput").ap()
    xTq = nc.dram_tensor("xTq", [C, SLOTS * P], BF16, kind="ExternalInput").ap()
    wqT = nc.dram_tensor("wqT", [C, C], BF16, kind="ExternalInput").ap()
    wkT = nc.dram_tensor("wkT", [C, C], BF16, kind="ExternalInput").ap()
    wvT = nc.dram_tensor("wvT", [C, C], BF16, kind="ExternalInput").ap()
    woT = nc.dram_tensor("woT", [C, C], BF16, kind="ExternalInput").ap()
    bq = nc.dram_tensor("bq", [P, C // P], F32, kind="ExternalInput").ap()
    bk = nc.dram_tensor("bk", [P, C // P], F32, kind="ExternalInput").ap()
    masks = nc.dram_tensor("masks", [SLOTS, 2, P, P], FP8, kind="ExternalInput").ap()
    ident = nc.dram_tensor("ident", [P, P], BF16, kind="ExternalInput").ap()
    y = nc.dram_tensor("y", [SLOTS * P, C], BF16, kind="ExternalOutput").ap()

    CB = C // P  # 8 column blocks of the channel dim

    with tile.TileContext(nc) as tc:
        with (
            tc.tile_pool(name="const", bufs=1) as cpool,
            tc.tile_pool(name="attn", bufs=1) as apool,
        ):
            masks_sb = cpool.tile([P, SLOTS, 2, P], FP8)
            ident_sb = cpool.tile([P, P], BF16)
            bq_sb = cpool.tile([P, CB], F32)
            bk_sb = cpool.tile([P, CB], F32)

            attn_out = apool.tile([P, SLOTS, C], BF16)

            with tc.tile_pool(name="qkv", bufs=1) as qkv:
                qT = qkv.tile([P, CB, SLOTS * P], BF16)
                kT = qkv.tile([P, CB, T], BF16)
                v = qkv.tile([P, NB, H * (DK + 1)], BF16)
                vg = v[:].rearrange("p t (h e) -> p t h e", e=DK + 1)
                nc.vector.memset(vg[:, :, :, DK : DK + 1], 1.0)

                # ---- Q projection (kb-major, 8 psum banks) ----
                with (
                    tc.tile_pool(name="xq", bufs=1) as xq_pool,
                    tc.tile_pool(name="wq", bufs=1) as wq_pool,
                    tc.tile_pool(name="pq", bufs=1, space="PSUM") as pq,
                ):
                    xq_sb = xq_pool.tile([P, CB, SLOTS * P], BF16)
                    xTq_r = xTq.rearrange("(ko p) t -> p ko t", p=P)
                    wq_sb = wq_pool.tile([P, CB, C], BF16)
                    wq_r = wqT.rearrange("(ko p) n -> p ko n", p=P)
                    # biases ride the idle Scalar engine's DMA queue so the
                    # first xq/wq pair issues immediately (PE starts earlier)
                    nc.scalar.dma_start(bq_sb[:], bq[:])
                    nc.scalar.dma_start(bk_sb[:], bk[:])
                    # startup is HBM-bw bound with all 8 cores loading at
                    # once: per-kb xq/wq pairs land just-in-time for the
                    # kb-major Q loop, so the PE starts ~7us earlier
                    for kb in range(CB):
                        nc.gpsimd.dma_start(xq_sb[:, kb, :], xTq_r[:, kb, :])
                        nc.gpsimd.dma_start(wq_sb[:, kb, :], wq_r[:, kb, :])
                    for nch in range(2):
                        acc = [
                            pq.tile([P, 512], F32, tag=f"qacc{cb}", name=f"qacc{cb}")
                            for cb in range(CB)
                        ]
                        for kb in range(CB):
                            for cb in range(CB):
                                nc.tensor.matmul(
                                    acc[cb][:],
                                    wq_sb[:, kb, ts(cb, P)],
                                    xq_sb[:, kb, ts(nch, 512)],
                                    start=(kb == 0),
                                    stop=(kb == CB - 1),
                                )
                        for cb in range(CB):
                            nc.vector.tensor_scalar_add(
                                qT[:, cb, ts(nch, 512)], acc[cb][:], bq_sb[:, cb : cb + 1]
                            )

                # ---- fused stream: K-proj / scores / exp / attnV / V-proj ----
                # Work is emitted as fine-grained units: score psum-groups
                # (Scalar-paced via exp) woven with K/V projection chains and
                # attnV chains, so the PE never idles long enough for the HAM
                # clock gate to re-throttle it, and the Scalar engine always
                # has the next score tile ready.
                with (
                    tc.tile_pool(name="xt", bufs=1) as xt_pool,
                    tc.tile_pool(name="wres", bufs=1) as wres,
                    tc.tile_pool(name="expS", bufs=2) as spool,
                    tc.tile_pool(name="small", bufs=4) as small,
                    tc.tile_pool(name="atp", bufs=1) as atp,
                    tc.tile_pool(name="pp", bufs=2, space="PSUM") as pp,
                    tc.tile_pool(name="ps_s", bufs=2, space="PSUM") as ps_s,
                    tc.tile_pool(name="ps_o", bufs=2, space="PSUM") as ps_o,
                ):
                    xT_sb = xt_pool.tile([P, CB, T], BF16)
                    wk_sb = wres.tile([P, CB, C], BF16, tag="wk", name="wk")
                    wv0_sb = wres.tile([P, CB, 512], BF16, tag="wv", name="wv0")
                    xT_r = xT.rearrange("(ko p) t -> p ko t", p=P)
                    wk_r = wkT.rearrange("(ko p) n -> p ko n", p=P)
                    wv_r = wvT.rearrange("(ko p) n -> p ko n", p=P)
                    # K-proj's first inputs go through the idle Scalar
                    # engine's DMA queue so they race the Q-phase loads
                    # instead of queuing behind them (startup is HBM-bw
                    # bound; the PE reaches K ~30us after Q starts).
                    nc.scalar.dma_start(wk_sb[:, :, 0:128], wk_r[:, :, 0:128])
                    nc.scalar.dma_start(xT_sb[:, :, 0:512], xT_r[:, :, 0:512])
                    nc.gpsimd.dma_start(xT_sb[:, :, 512:1024], xT_r[:, :, 512:1024])
                    nc.gpsimd.dma_start(wk_sb[:, :, 128:512], wk_r[:, :, 128:512])
                    nc.gpsimd.dma_start(wk_sb[:, :, 512:1024], wk_r[:, :, 512:1024])
                    nc.gpsimd.dma_start(
                        xT_sb[:, :, 1024:2048], xT_r[:, :, 1024:2048]
                    )
                    nc.gpsimd.dma_start(
                        masks_sb[:], masks[:].rearrange("s t p q -> p s t q")
                    )
                    nc.gpsimd.dma_start(ident_sb[:], ident[:])
                    nc.gpsimd.dma_start(wv0_sb[:], wv_r[:, :, 0:512])

                    exp_tiles = {}

                    def u_kchain(cb, nch):
                        def emit():
                            acc = pp.tile([P, 512], F32, tag="pp")
                            for kb in range(CB):
                                nc.tensor.matmul(
                                    acc[:],
                                    wk_sb[:, kb, ts(cb, P)],
                                    xT_sb[:, kb, ts(nch, 512)],
                                    start=(kb == 0),
                                    stop=(kb == CB - 1),
                                )
                            nc.vector.tensor_scalar_add(
                                kT[:, cb, ts(nch, 512)], acc[:], bk_sb[:, cb : cb + 1]
                            )
                        return emit

                    def u_vchain(tb, dch, wv_half):
                        def emit():
                            acc = pp.tile([P, 512], F32, tag="pp")
                            for kb in range(CB):
                                nc.tensor.matmul(
                                    acc[:],
                                    xT_sb[:, kb, ts(tb, P)],
                                    wv_half[:, kb, :],
                                    start=(kb == 0),
                                    stop=(kb == CB - 1),
                                )
                            nc.vector.tensor_copy(
                                vg[:, tb, dch * 8 : (dch + 1) * 8, 0:DK],
                                acc[:].rearrange("p (h e) -> p h e", e=DK),
                            )
                        return emit

                    def u_transpose(cb):
                        # real late-slot filler: transpose the finished
                        # head-pair column of attn_out into aT (PSUM via the
                        # shared pp tag, bitcast to bf16)
                        def emit():
                            for s in range(SLOTS):
                                pst = pp.tile([P, 512], F32, tag="pp", name="pst")
                                pb = pst[:, 0:64].bitcast(BF16)
                                nc.tensor.transpose(
                                    pb, attn_out[:, s, ts(cb, P)], ident_sb[:]
                                )
                                nc.vector.tensor_copy(aT[:, cb, ts(s, P)], pb)
                        return emit

                    def u_score(h, jb):
                        # one jb-pair group: matmuls + exp (+ causal mask)
                        hp = (h % 2) * DK
                        cbh = h // 2
                        w = W_JB[jb]
                        q0 = P * (jb // 2)
                        s = jb // 2

                        def emit():
                            expS = exp_tiles[h]
                            if jb >= 8:
                                pss = ps_s.tile([P, SLOTS * P], F32, tag="pss")
                                for i in range(2):
                                    nc.tensor.matmul(
                                        pss[:, 512 * i : 512 * i + w],
                                        kT[hp : hp + DK, cbh, ts(jb + i, P)],
                                        qT[hp : hp + DK, cbh, q0:],
                                        start=True,
                                        stop=True,
                                    )
                                nc.scalar.activation(
                                    expS[
                                        :, OFF_JB[jb] : OFF_JB[jb] + 2 * w
                                    ].rearrange("p (t q) -> p t q", t=2),
                                    pss[:].rearrange("p (t c) -> p t c", t=2)[
                                        :, :, 0:w
                                    ],
                                    EXP,
                                    scale=float(SCALE),
                                )
                            else:
                                for i in range(2):
                                    pss = ps_s.tile([P, SLOTS * P], F32, tag="pss")
                                    for aa, bb in ((q0, 512), (512, SLOTS * P)):
                                        nc.tensor.matmul(
                                            pss[:, aa:bb],
                                            kT[hp : hp + DK, cbh, ts(jb + i, P)],
                                            qT[hp : hp + DK, cbh, aa:bb],
                                            start=True,
                                            stop=True,
                                        )
                                    nc.scalar.activation(
                                        expS[:, OFF_JB[jb + i] : OFF_JB[jb + i] + w],
                                        pss[:, q0:],
                                        EXP,
                                        scale=float(SCALE),
                                    )
                            blk = expS[:, OFF_JB[jb] : OFF_JB[jb] + 2 * w].rearrange(
                                "p (t q) -> p t q", t=2
                            )[:, :, 0:P]
                            nc.vector.tensor_mul(blk, blk, masks_sb[:, s, :, :])
                        return emit

                    def u_attnv(h):
                        # chains for slots s and s+4 interleave across the
                        # two pso banks so consecutive matmuls never RMW the
                        # same PSUM bank back-to-back
                        def emit():
                            expS = exp_tiles[h]
                            pso0 = ps_o.tile([P, 4, DK + 1], F32, tag="pso")
                            pso1 = ps_o.tile([P, 4, DK + 1], F32, tag="pso")
                            vcol = v[:, :, h * (DK + 1) : (h + 1) * (DK + 1)]
                            for si in range(4):
                                sA, sB = si, si + 4
                                JA, JB = 2 * sA + 2, 2 * sB + 2
                                for jb in range(JB):
                                    if jb < JA:
                                        o = OFF_JB[jb] + sA * P - P * (jb // 2)
                                        nc.tensor.matmul(
                                            pso0[:, si, :],
                                            expS[:, o : o + P],
                                            vcol[:, jb, :],
                                            start=(jb == 0),
                                            stop=(jb == JA - 1),
                                        )
                                    o = OFF_JB[jb] + sB * P - P * (jb // 2)
                                    nc.tensor.matmul(
                                        pso1[:, si, :],
                                        expS[:, o : o + P],
                                        vcol[:, jb, :],
                                        start=(jb == 0),
                                        stop=(jb == JB - 1),
                                    )
                            for g, pso in ((0, pso0), (1, pso1)):
                                rec = small.tile([P, 4], F32, tag="rec")
                                nc.vector.reciprocal(
                                    rec[:],
                                    pso[:, :, DK : DK + 1].rearrange(
                                        "p s o -> p (s o)"
                                    ),
                                )
                                for si in range(4):
                                    s = 4 * g + si
                                    nc.vector.tensor_scalar_mul(
                                        attn_out[:, s, h * DK : (h + 1) * DK],
                                        pso[:, si, 0:DK],
                                        rec[:, si : si + 1],
                                    )
                        return emit

                    def new_head(h):
                        exp_tiles[h] = spool.tile([P, NPACK], BF16, tag="expS", name="expS")

                    # V chains: dch0 fills slot 0 (A(0) needs it at
                    # slot 1), dch1 fills slots 2-4 (deadline: A(8) at slot
                    # 5; slot 1 left light so the wv1 load can land).
                    # Slots 5-7 are filled with the attn_out transposes.
                    aT = atp.tile([P, CB, SLOTS * P], BF16)
                    vfill = {0: [u_vchain(tb, 0, wv0_sb) for tb in range(NB)]}

                    for c in range(CB):
                        h0, h1 = 2 * c, 2 * c + 1
                        if c == 1:
                            wv1_sb = wres.tile(
                                [P, CB, 512], BF16, tag="wv", name="wv1"
                            )
                            nc.gpsimd.dma_start(wv1_sb[:], wv_r[:, :, 512:1024])
                            vfill[2] = [u_vchain(tb, 1, wv1_sb) for tb in range(6)]
                            vfill[3] = [u_vchain(tb, 1, wv1_sb) for tb in range(6, 11)]
                            vfill[4] = [u_vchain(tb, 1, wv1_sb) for tb in range(11, 16)]
                            vfill[5] = [u_transpose(cb) for cb in range(4)]
                            vfill[6] = [u_transpose(4)]
                            vfill[7] = [u_transpose(5)]
                        fills = list(vfill.get(c, []))
                        nf = len(fills)
                        fi = 0
                        if c >= 1:
                            u_attnv(h0 - 2)()
                        new_head(h0)
                        for p in range(8):  # jb-pair index
                            if p % 2 == 0:
                                u_kchain(c, p // 2)()
                            u_score(h0, 2 * p)()
                            while fi * 16 < (p + 1) * nf:
                                fills[fi]()
                                fi += 1
                        if c >= 1:
                            u_attnv(h0 - 1)()
                        new_head(h1)
                        for p in range(8):
                            u_score(h1, 2 * p)()
                            while fi * 16 < (p + 9) * nf:
                                fills[fi]()
                                fi += 1
                        for f in fills[fi:]:
                            f()
                    u_attnv(H - 2)()
                    u_transpose(6)()
                    woT_sb = wres.tile([P, CB, C], BF16, tag="wk", name="woT")
                    woT_r = woT.rearrange("(ko p) n -> p ko n", p=P)
                    nc.gpsimd.dma_start(woT_sb[:, :, 0:512], woT_r[:, :, 0:512])
                    nc.gpsimd.dma_start(
                        woT_sb[:, :, 512:1024], woT_r[:, :, 512:1024]
                    )
                    u_attnv(H - 1)()
                    u_transpose(7)()

                    # ---- output projection, staged through the (now dead)
                    # attn_out tile in bf16; y is written back as bf16 ----
                    y_r = y.rearrange("(tb p) c -> p tb c", p=P)
                    for tb in range(SLOTS):
                        for nch in range(2):
                            psy = pp.tile([P, 512], F32, tag="pp", name="psy")
                            for cbk in range(CB):
                                nc.tensor.matmul(
                                    psy[:],
                                    aT[:, cbk, ts(tb, P)],
                                    woT_sb[:, cbk, ts(nch, 512)],
                                    start=(cbk == 0),
                                    stop=(cbk == CB - 1),
                                )
                            st = attn_out[:, tb, 512 * nch : 512 * nch + 512]
                            nc.vector.tensor_copy(st, psy[:])
                            nc.gpsimd.dma_start(y_r[:, tb, ts(nch, 512)], st)

    nc.compile()
    return nc


def _host_inputs(x, mask, Wq, bq_v, Wk, bk_v, Wv, bv_v, Wo, bo_v):
    """Per-core input maps + the host-side output bias correction."""
    f32 = np.float32
    bf16 = ml_dtypes.bfloat16
    wqT = np.ascontiguousarray(np.asarray(Wq, f32).T).astype(bf16)
    wkT = np.ascontiguousarray(np.asarray(Wk, f32).T).astype(bf16)
    wvT = np.ascontiguousarray(np.asarray(Wv, f32).T).astype(bf16)
    woT = np.ascontiguousarray(np.asarray(Wo, f32).T).astype(bf16)
    bq_p = np.ascontiguousarray(np.asarray(bq_v, f32).reshape(C // P, P).T)
    bk_p = np.ascontiguousarray(np.asarray(bk_v, f32).reshape(C // P, P).T)
    identity = np.eye(P, dtype=f32).astype(bf16)
    # exact v/o bias fold: softmax rows sum to 1, so v+bv adds bv to attn out
    bo_eff = (np.asarray(bo_v, f32) + np.asarray(bv_v, f32) @ np.asarray(Wo, f32).T)

    # per-half causal boundary masks for the last two key blocks of each slot
    mask_half = []
    tri = np.tril(np.ones((P, P), f32)).T  # [j, i] = 1 where j <= i
    for half in range(2):
        m = np.zeros((SLOTS, 2, P, P), f32)
        for s in range(SLOTS):
            g = QBLKS[half][s]
            for idx, jb in enumerate((2 * s, 2 * s + 1)):
                if jb < g:
                    m[s, idx] = 1.0
                elif jb == g:
                    m[s, idx] = tri
        mask_half.append(m.astype(ml_dtypes.float8_e4m3fn))

    xn = np.asarray(x, f32)
    in_maps = []
    for core in range(8):
        b, half = divmod(core, 2)
        xT = np.ascontiguousarray(xn[b].T).astype(bf16)
        qtok = np.concatenate([np.arange(g * P, (g + 1) * P) for g in QBLKS[half]])
        xTq = np.ascontiguousarray(xn[b][qtok].T).astype(bf16)
        in_maps.append(
            {
                "xT": xT,
                "xTq": xTq,
                "wqT": wqT,
                "wkT": wkT,
                "wvT": wvT,
                "woT": woT,
                "bq": bq_p,
                "bk": bk_p,
                "masks": mask_half[half],
                "ident": identity,
            }
        )
    return in_maps, bo_eff


def _run(inputs, trace=False):
    if "nc" not in _cache:
        _cache["nc"] = _build()
    nc = _cache["nc"]
    in_maps, bo_eff = _host_inputs(
        inputs["x"], inputs["mask"],
        inputs["Wq"], inputs["bq"], inputs["Wk"], inputs["bk"],
        inputs["Wv"], inputs["bv"], inputs["Wo"], inputs["bo"],
    )
    res = run_bass_kernel_spmd(nc, in_maps, list(range(8)), trace=trace)
    out = np.empty((B, T, C), np.float32)
    for core in range(8):
        b, half = divmod(core, 2)
        yc = np.asarray(res.results[core]["y"], dtype=np.float32)
        for s, g in enumerate(QBLKS[half]):
            out[b, g * P : (g + 1) * P] = yc[s * P : (s + 1) * P]
    out += bo_eff
    return out, res


def kernel(**inputs):
    out, _ = _run(inputs, trace=False)
    return out

